# revision 16
# baseline (speedup 1.0000x reference)
"""Multi-head attention (MockCoreAttention) for 8 Trainium2 NeuronCores.

Problem: q,k,v [s=2048, b=2, n=16, d=128] fp32 ->
         out = softmax(q@k^T/sqrt(d)) @ v reshaped to [s, b, n*d].

Strategy (head parallel): 32 (b,n) heads sharded 4-per-core across 8 cores.
Per head, on-device:
  S^T[sk, sq] = K Q^T computed in 16 sk-chunks of 128 (TensorE, bf16 in,
  fp32 PSUM).  Q is pre-scaled on host by softmax_scale*log2(e), so the
  PSUM scores are z = log2(e^{s*scale}) and the softmax numerator is 2^z.
  exp: most chunks via ScalarE ACTIVATE Exp with scale=ln2 (exact), a
  subset via a single DVE tensor_scalar implementing the Schraudolph
  bit-trick: bf16bits = rint(z*128 + 16250.4) -- the int16 bit pattern IS
  bf16(2^z) to ~2% rms.  This splits the exp load across two engines.
  No max-subtraction pass: scores are ~N(0,1) so exp cannot overflow.
  O^T[d, sq] += V_j^T P_j^T accumulated over chunks in PSUM (TensorE).
  rowsum[q]: chain-add the P^T tiles on DVE (bf16, whole [128,ef*sq]
  slabs), one transient ones-matmul per group broadcasts the sums across
  partitions; normalization = fast reciprocal + elementwise multiply in
  O^T layout (DVE), output DMA in bf16.
Host side does layout-only work: head sharding, [s,d]->[d,s] transposes,
bf16 cast + Q pre-scale, and the final gather/cast/reshape.
"""

import numpy as np
import ml_dtypes

import sys

for _p in ("/opt/trn_rl_repo",):
    if _p not in sys.path:
        sys.path.append(_p)

S, B, NH, D = 2048, 2, 16, 128
H = B * NH            # 32 total heads
NCORES = 8
HL = H // NCORES      # 4 heads per core
SOFTMAX_SCALE = 0.08838834764831845  # 1/sqrt(128)
LOG2E = 1.4426950408889634
LN2 = 0.6931471805599453
SCHRAU_B = 16256.0 - 5.6  # tuned: rint, max rel err 3.26%, rms 2.0%

BF16 = ml_dtypes.bfloat16


def build_program(s=S, hl=HL, sq=512, nmm=512, repeat=1,
                  stages=("qk", "exp", "pv", "sums", "tail"), lookahead=2,
                  s_bufs=3, pt_bufs=8, o_bufs=2, exp_fuse=2,
                  fs_bufs=5, in_bufs=2, tail_bufs=3,
                  dve_exp=1, recip="fast", out_dtype="bf16", slab_adds=1,
                  sum_fold=0, sm_pool=0, gps_adds=0, add_split=1,
                  qk_same_w=0):
    """Build the per-core Bass/Tile program (SPMD: identical on all cores).

    sq: q-columns per group; PSUM budget: s_bufs*exp_fuse*sq + o_bufs*sq
    <= 4096-512 fp32 per partition (8 banks of 512).
    dve_exp: how many of the (j_chunks/exp_fuse) S^T tiles per group get
    their exp on DVE (Schraudolph) instead of ScalarE.
    """
    import concourse.tile as tile
    from concourse import bacc, mybir

    j_chunks = s // 128
    groups = s // sq
    assert sq % nmm == 0
    nsub = sq // nmm
    assert j_chunks % exp_fuse == 0
    jj_units = j_chunks // exp_fuse
    assert 0 <= dve_exp <= jj_units
    # spread the DVE-exp'd tiles evenly through the chunk sequence
    dve_jjs = {int((i + 0.5) * jj_units / dve_exp) for i in range(dve_exp)}

    dt_in = mybir.dt.bfloat16
    np_in = BF16
    out_dt = {"bf16": mybir.dt.bfloat16, "fp32": mybir.dt.float32}[out_dtype]

    nc = bacc.Bacc("TRN2", target_bir_lowering=False, debug=False,
                   enable_asserts=False)

    qt = nc.dram_tensor("qt", [hl, D, s], dt_in, kind="ExternalInput").ap()
    kt = nc.dram_tensor("kt", [hl, D, s], dt_in, kind="ExternalInput").ap()
    vp = nc.dram_tensor("vp", [hl, 128, j_chunks, D], dt_in,
                        kind="ExternalInput").ap()
    outT = nc.dram_tensor("outT", [hl, D, s], out_dt,
                          kind="ExternalOutput").ap()

    f32 = mybir.dt.float32
    i16 = mybir.dt.int16
    Exp = mybir.ActivationFunctionType.Exp
    Alu = mybir.AluOpType

    with tile.TileContext(nc) as tc:
        with (
            tc.tile_pool(name="singles", bufs=1) as singles,
            tc.tile_pool(name="qin", bufs=in_bufs) as qin,
            tc.tile_pool(name="kin", bufs=in_bufs) as kin,
            tc.tile_pool(name="vin", bufs=in_bufs) as vin,
            tc.tile_pool(name="pt", bufs=pt_bufs) as ptp,
            tc.tile_pool(name="spsum", bufs=s_bufs, space="PSUM") as sp,
            tc.tile_pool(name="opsum", bufs=o_bufs, space="PSUM") as op,
            tc.tile_pool(name="smpsum", bufs=max(1, sm_pool),
                         space="PSUM") as smp,
            tc.tile_pool(name="outsb", bufs=tail_bufs) as outsb,
            tc.tile_pool(name="recip", bufs=tail_bufs) as rcp,
            tc.tile_pool(name="fsum", bufs=fs_bufs) as fsp,
            tc.tile_pool(name="gsum", bufs=3) as gsp,
        ):
            ones = singles.tile([128, 128], dt_in)
            nc.vector.memset(ones[:], 1.0)

            def body(_it=None):
                head_tiles = {}
                group_psum = {}
                fold_state = {}

                def ensure_head(h):
                    if h not in head_tiles:
                        qt_t = qin.tile([D, s], dt_in)
                        nc.sync.dma_start(qt_t[:], qt[h, :, :])
                        kt_t = kin.tile([D, s], dt_in)
                        nc.sync.dma_start(kt_t[:], kt[h, :, :])
                        vp_t = vin.tile([128, j_chunks, D], dt_in)
                        nc.sync.dma_start(vp_t[:], vp[h, :, :, :])
                        head_tiles[h] = (qt_t, kt_t, vp_t)
                    return head_tiles[h]

                def emit_qk(h, g, jj):
                    qt_t, kt_t, _ = ensure_head(h)
                    s_t = sp.tile([128, exp_fuse, sq], f32)
                    if "qk" in stages:
                        for u in range(exp_fuse):
                            j = (0 if qk_same_w else jj) * exp_fuse + \
                                (0 if qk_same_w else u)
                            for c in range(nsub):
                                nc.tensor.matmul(
                                    s_t[:, u, c * nmm:(c + 1) * nmm],
                                    lhsT=kt_t[:, j * 128:(j + 1) * 128],
                                    rhs=qt_t[:, g * sq + c * nmm:
                                             g * sq + (c + 1) * nmm],
                                )
                    return s_t

                def emit_rest(h, g, jj, s_t):
                    _, _, vp_t = head_tiles[h]
                    if (h, g) not in group_psum:
                        group_psum[(h, g)] = op.tile([D, sq], f32, name="o_t")
                    o_t = group_psum[(h, g)]
                    first_t, last_t = (jj == 0), (jj == jj_units - 1)
                    pt_t = ptp.tile([128, exp_fuse, sq], dt_in)
                    if "exp" in stages:
                        if jj in dve_jjs:
                            nc.vector.tensor_scalar(
                                pt_t[:].bitcast(i16), s_t[:],
                                128.0, SCHRAU_B, Alu.mult, Alu.add)
                        else:
                            nc.scalar.activation(pt_t[:], s_t[:], Exp,
                                                 scale=LN2)
                    if "pv" in stages:
                        for u in range(exp_fuse):
                            j = jj * exp_fuse + u
                            for c in range(nsub):
                                cs = slice(c * nmm, (c + 1) * nmm)
                                nc.tensor.matmul(
                                    o_t[:, cs], lhsT=vp_t[:, j, :],
                                    rhs=pt_t[:, u, cs],
                                    start=(j == 0), stop=(j == j_chunks - 1))
                    if "sums" in stages:
                        if slab_adds and sum_fold == 0:
                            if first_t:
                                fold_state[(h, g)] = pt_t[:]
                            else:
                                t = fsp.tile([128, exp_fuse, sq], dt_in,
                                             name="fs")
                                nc.vector.tensor_add(t[:], fold_state[(h, g)],
                                                     pt_t[:])
                                fold_state[(h, g)] = t[:]
                            if last_t:
                                folded = fold_state.pop((h, g))
                                if exp_fuse == 1:
                                    ff = folded
                                else:
                                    fft = fsp.tile([128, sq], dt_in,
                                                   name="ff")
                                    acc = folded[:, 0, :]
                                    for u in range(1, exp_fuse):
                                        nc.vector.tensor_add(
                                            fft[:], acc, folded[:, u, :])
                                        acc = fft[:]
                                    ff = acc
                                if sm_pool:
                                    sm_t = smp.tile([128, sq], f32,
                                                    name="sm_t")
                                    sm_ap = sm_t[:]
                                else:
                                    sm_t = sp.tile([128, exp_fuse, sq], f32,
                                                   name="sm_t", tag="s_t")
                                    sm_ap = sm_t[:, 0, :]
                                for c in range(nsub):
                                    cs = slice(c * nmm, (c + 1) * nmm)
                                    nc.tensor.matmul(
                                        sm_ap[:, cs], lhsT=ones[:],
                                        rhs=ff[:, cs], start=True, stop=True)
                                group_psum[(h, g)] = (o_t, sm_ap)
                        elif sum_fold == 0:
                            gps_start = (j_chunks - (gps_adds + 1)
                                         if gps_adds else j_chunks)
                            for u in range(exp_fuse):
                                j = jj * exp_fuse + u
                                if j >= gps_start:
                                    key = (h, g, "gps")
                                    if j == gps_start:
                                        fold_state[key] = pt_t[:, u, :]
                                    else:
                                        acc = fold_state[key]
                                        t = gsp.tile([128, sq], dt_in,
                                                     name="gs")
                                        nc.gpsimd.tensor_add(t[:], acc,
                                                             pt_t[:, u, :])
                                        fold_state[key] = t[:]
                                elif j == 0:
                                    fold_state[(h, g)] = pt_t[:, u, :]
                                else:
                                    acc = fold_state[(h, g)]
                                    t = fsp.tile([128, sq], dt_in, name="fs")
                                    sw = sq // add_split
                                    for a in range(add_split):
                                        asl = slice(a * sw, (a + 1) * sw)
                                        nc.vector.tensor_add(
                                            t[:, asl], acc[:, asl],
                                            pt_t[:, u, asl])
                                    fold_state[(h, g)] = t[:]
                            if last_t:
                                facc = fold_state.pop((h, g))
                                if gps_adds:
                                    gacc = fold_state.pop((h, g, "gps"))
                                    t = fsp.tile([128, sq], dt_in, name="fs")
                                    nc.vector.tensor_add(t[:], facc, gacc)
                                    facc = t[:]
                                if sm_pool:
                                    sm_t = smp.tile([128, sq], f32,
                                                    name="sm_t")
                                    sm_ap = sm_t[:]
                                else:
                                    sm_t = sp.tile([128, exp_fuse, sq], f32,
                                                   name="sm_t", tag="s_t")
                                    sm_ap = sm_t[:, 0, :]
                                for c in range(nsub):
                                    cs = slice(c * nmm, (c + 1) * nmm)
                                    nc.tensor.matmul(
                                        sm_ap[:, cs], lhsT=ones[:],
                                        rhs=facc[:, cs],
                                        start=True, stop=True)
                                group_psum[(h, g)] = (o_t, sm_ap)
                    if last_t and "tail" in stages:
                        o_t, sm_t = group_psum[(h, g)]
                        recip_t = rcp.tile([128, sq], f32)
                        if recip == "fast":
                            nc.vector.reciprocal_approx_fast(recip_t[:],
                                                             sm_t[:])
                        else:
                            nc.vector.reciprocal(recip_t[:], sm_t[:])
                        otn = outsb.tile([D, sq], out_dt)
                        nc.vector.tensor_mul(otn[:], o_t[:], recip_t[:])
                        nc.sync.dma_start(
                            outT[h, :, g * sq:(g + 1) * sq], otn[:])
                        del group_psum[(h, g)]

                chunk_list = [(h, g, jj) for h in range(hl)
                              for g in range(groups)
                              for jj in range(jj_units)]
                pending = []
                for ch in chunk_list:
                    pending.append((ch, emit_qk(*ch)))
                    if len(pending) > lookahead:
                        (h, g, j), s_t = pending.pop(0)
                        emit_rest(h, g, j, s_t)
                while pending:
                    (h, g, j), s_t = pending.pop(0)
                    emit_rest(h, g, j, s_t)

            if repeat == 1:
                body()
            else:
                with tc.For_i(0, repeat, 1) as _i:
                    body(_i)

    nc.compile()
    return nc, np_in


def shard_inputs(q, k, v, s=S, hl=HL, ncores=NCORES, np_in=BF16):
    """Host-side layout prep: per-core per-head transposed views, cast.
    Q is pre-scaled by softmax_scale*log2(e) (see build_program)."""
    nheads = ncores * hl
    j_chunks = s // 128
    # [s,b,n,d] -> [b,n,d,s] -> [H, d, s]
    qt = np.ascontiguousarray(
        q.transpose(1, 2, 3, 0) * np.float32(SOFTMAX_SCALE * LOG2E)
    ).reshape(nheads, D, s)
    kt = np.ascontiguousarray(k.transpose(1, 2, 3, 0)).reshape(nheads, D, s)
    # [s,b,n,d] -> [b,n,s,d] -> [H, J, 128, d] -> [H, 128, J, d]
    vpm = (v.transpose(1, 2, 0, 3)
            .reshape(nheads, j_chunks, 128, D)
            .transpose(0, 2, 1, 3))
    qt = qt.astype(np_in)
    kt = kt.astype(np_in)
    vpm = np.ascontiguousarray(vpm).astype(np_in)
    in_maps = []
    for c in range(ncores):
        sl = slice(c * hl, (c + 1) * hl)
        in_maps.append({
            "qt": np.ascontiguousarray(qt[sl]),
            "kt": np.ascontiguousarray(kt[sl]),
            "vp": np.ascontiguousarray(vpm[sl]),
        })
    return in_maps


def gather_output(results, s=S, hl=HL, ncores=NCORES):
    """[{outT: [hl, D, s]}] per core -> full [s, B, NH*D] fp32."""
    outT = np.stack([np.asarray(r["outT"]).astype(np.float32)
                     for r in results])                        # [C, hl, D, s]
    out_heads = outT.reshape(ncores * hl, D, s)                # [H, D, s]
    out = out_heads.transpose(2, 0, 1)                         # [s, H, D]
    return np.ascontiguousarray(out).reshape(s, B, NH * D)


_CACHE = {}

# Measured (paired A/B on HW, reps=1 vs 4097 wall-differencing):
#   - recip="fast" (RECIPROCAL_APPROX_FAST, ~51 ULP) vs bit-exact
#     reciprocal (~6 cyc/elem): neutral-to-positive, strictly less DVE.
#   - out_dtype="bf16": -25 us/iter (halves output DMA + 16-bit writes).
#   - slab_adds / gps_adds (GpSimd offload) / dve_exp (Schraudolph exp on
#     DVE) / sq=1024 / lookahead 1 or 3 / bigger pools: all regressions.
BEST_CFG = dict(sq=512, nmm=512, lookahead=2, s_bufs=3, o_bufs=2,
                exp_fuse=2, pt_bufs=8, tail_bufs=3, fs_bufs=5, in_bufs=3,
                dve_exp=0, recip="fast", out_dtype="bf16", slab_adds=0)


def _get_program(**cfg):
    key = tuple(sorted((k, str(v)) for k, v in cfg.items()))
    if key not in _CACHE:
        _CACHE[key] = build_program(**cfg)
    return _CACHE[key]


def run(q, k, v, trace=False, **cfg):
    """Run on the 8 NeuronCores; returns (out, BassKernelResults)."""
    from concourse.bass_utils import run_bass_kernel_spmd

    full_cfg = {**BEST_CFG, **cfg}
    nc, np_in = _get_program(**full_cfg)
    in_maps = shard_inputs(q, k, v, np_in=np_in)
    res = run_bass_kernel_spmd(nc, in_maps, core_ids=list(range(NCORES)),
                               trace=trace)
    return gather_output(res.results), res


def kernel(q, k, v):
    q = np.asarray(q, dtype=np.float32)
    k = np.asarray(k, dtype=np.float32)
    v = np.asarray(v, dtype=np.float32)
    out, _ = run(q, k, v)
    return out


# revision 18
# speedup vs baseline: 1.1781x; 1.1781x over previous
"""Multi-head attention (MockCoreAttention) for 8 Trainium2 NeuronCores.

Problem: q,k,v [s=2048, b=2, n=16, d=128] fp32 ->
         out = softmax(q@k^T/sqrt(d)) @ v reshaped to [s, b, n*d].

Strategy (head parallel): 32 (b,n) heads sharded 4-per-core across 8 cores.
Per head, on-device:
  S^T[sk, sq] = K Q^T computed in 16 sk-chunks of 128 (TensorE, bf16 in,
  fp32 PSUM).  Q is pre-scaled on host by softmax_scale*log2(e), so the
  PSUM scores are z = log2(e^{s*scale}) and the softmax numerator is 2^z.
  exp: most chunks via ScalarE ACTIVATE Exp with scale=ln2 (exact), a
  subset via a single DVE tensor_scalar implementing the Schraudolph
  bit-trick: bf16bits = rint(z*128 + 16250.4) -- the int16 bit pattern IS
  bf16(2^z) to ~2% rms.  This splits the exp load across two engines.
  No max-subtraction pass: scores are ~N(0,1) so exp cannot overflow.
  O^T[d, sq] += V_j^T P_j^T accumulated over chunks in PSUM (TensorE).
  rowsum[q]: chain-add the P^T tiles on DVE (bf16, whole [128,ef*sq]
  slabs), one transient ones-matmul per group broadcasts the sums across
  partitions; normalization = fast reciprocal + elementwise multiply in
  O^T layout (DVE), output DMA in bf16.
Host side does layout-only work: head sharding, [s,d]->[d,s] transposes,
bf16 cast + Q pre-scale, and the final gather/cast/reshape.
"""

import numpy as np
import ml_dtypes

import sys

for _p in ("/opt/trn_rl_repo",):
    if _p not in sys.path:
        sys.path.append(_p)

S, B, NH, D = 2048, 2, 16, 128
H = B * NH            # 32 total heads
NCORES = 8
HL = H // NCORES      # 4 heads per core
SOFTMAX_SCALE = 0.08838834764831845  # 1/sqrt(128)
LOG2E = 1.4426950408889634
LN2 = 0.6931471805599453
SCHRAU_B = 16256.0 - 5.6  # tuned: rint, max rel err 3.26%, rms 2.0%

BF16 = ml_dtypes.bfloat16


def build_program(s=S, hl=HL, sq=512, nmm=512, repeat=1,
                  stages=("qk", "exp", "pv", "sums", "tail"), lookahead=2,
                  s_bufs=3, pt_bufs=8, o_bufs=2, exp_fuse=2,
                  fs_bufs=5, in_bufs=2, tail_bufs=3,
                  dve_exp=1, recip="fast", out_dtype="bf16", slab_adds=1,
                  sum_fold=0, sm_pool=0, gps_adds=0, add_split=1,
                  qk_same_w=0, hp_dma=0):
    """Build the per-core Bass/Tile program (SPMD: identical on all cores).

    sq: q-columns per group; PSUM budget: s_bufs*exp_fuse*sq + o_bufs*sq
    <= 4096-512 fp32 per partition (8 banks of 512).
    dve_exp: how many of the (j_chunks/exp_fuse) S^T tiles per group get
    their exp on DVE (Schraudolph) instead of ScalarE.
    """
    import concourse.tile as tile
    from concourse import bacc, mybir

    j_chunks = s // 128
    groups = s // sq
    assert sq % nmm == 0
    nsub = sq // nmm
    assert j_chunks % exp_fuse == 0
    jj_units = j_chunks // exp_fuse
    assert 0 <= dve_exp <= jj_units
    # spread the DVE-exp'd tiles evenly through the chunk sequence
    dve_jjs = {int((i + 0.5) * jj_units / dve_exp) for i in range(dve_exp)}

    dt_in = mybir.dt.bfloat16
    np_in = BF16
    out_dt = {"bf16": mybir.dt.bfloat16, "fp32": mybir.dt.float32}[out_dtype]

    nc = bacc.Bacc("TRN2", target_bir_lowering=False, debug=False,
                   enable_asserts=False)

    qt = nc.dram_tensor("qt", [hl, D, s], dt_in, kind="ExternalInput").ap()
    kt = nc.dram_tensor("kt", [hl, D, s], dt_in, kind="ExternalInput").ap()
    vp = nc.dram_tensor("vp", [hl, 128, j_chunks, D], dt_in,
                        kind="ExternalInput").ap()
    outT = nc.dram_tensor("outT", [hl, D, s], out_dt,
                          kind="ExternalOutput").ap()

    f32 = mybir.dt.float32
    i16 = mybir.dt.int16
    Exp = mybir.ActivationFunctionType.Exp
    Alu = mybir.AluOpType

    with tile.TileContext(nc) as tc:
        with (
            tc.tile_pool(name="singles", bufs=1) as singles,
            tc.tile_pool(name="qin", bufs=in_bufs) as qin,
            tc.tile_pool(name="kin", bufs=in_bufs) as kin,
            tc.tile_pool(name="vin", bufs=in_bufs) as vin,
            tc.tile_pool(name="pt", bufs=pt_bufs) as ptp,
            tc.tile_pool(name="spsum", bufs=s_bufs, space="PSUM") as sp,
            tc.tile_pool(name="opsum", bufs=o_bufs, space="PSUM") as op,
            tc.tile_pool(name="smpsum", bufs=max(1, sm_pool),
                         space="PSUM") as smp,
            tc.tile_pool(name="outsb", bufs=tail_bufs) as outsb,
            tc.tile_pool(name="recip", bufs=tail_bufs) as rcp,
            tc.tile_pool(name="fsum", bufs=fs_bufs) as fsp,
            tc.tile_pool(name="gsum", bufs=3) as gsp,
        ):
            ones = singles.tile([128, 128], dt_in)
            nc.vector.memset(ones[:], 1.0)

            def body(_it=None):
                head_tiles = {}
                group_psum = {}
                fold_state = {}

                def ensure_head(h):
                    if h not in head_tiles:
                        import contextlib
                        hp = (tc.high_priority() if hp_dma
                              else contextlib.nullcontext())
                        with hp:
                            qt_t = qin.tile([D, s], dt_in)
                            nc.sync.dma_start(qt_t[:], qt[h, :, :])
                            kt_t = kin.tile([D, s], dt_in)
                            nc.sync.dma_start(kt_t[:], kt[h, :, :])
                            vp_t = vin.tile([128, j_chunks, D], dt_in)
                            nc.sync.dma_start(vp_t[:], vp[h, :, :, :])
                        head_tiles[h] = (qt_t, kt_t, vp_t)
                    return head_tiles[h]

                def emit_qk(h, g, jj):
                    qt_t, kt_t, _ = ensure_head(h)
                    s_t = sp.tile([128, exp_fuse, sq], f32)
                    if "qk" in stages:
                        for u in range(exp_fuse):
                            j = (0 if qk_same_w else jj) * exp_fuse + \
                                (0 if qk_same_w else u)
                            for c in range(nsub):
                                nc.tensor.matmul(
                                    s_t[:, u, c * nmm:(c + 1) * nmm],
                                    lhsT=kt_t[:, j * 128:(j + 1) * 128],
                                    rhs=qt_t[:, g * sq + c * nmm:
                                             g * sq + (c + 1) * nmm],
                                )
                    return s_t

                def emit_rest(h, g, jj, s_t):
                    _, _, vp_t = head_tiles[h]
                    if (h, g) not in group_psum:
                        group_psum[(h, g)] = op.tile([D, sq], f32, name="o_t")
                    o_t = group_psum[(h, g)]
                    first_t, last_t = (jj == 0), (jj == jj_units - 1)
                    pt_t = ptp.tile([128, exp_fuse, sq], dt_in)
                    if "exp" in stages:
                        if jj in dve_jjs:
                            nc.vector.tensor_scalar(
                                pt_t[:].bitcast(i16), s_t[:],
                                128.0, SCHRAU_B, Alu.mult, Alu.add)
                        else:
                            nc.scalar.activation(pt_t[:], s_t[:], Exp,
                                                 scale=LN2)
                    if "pv" in stages:
                        for u in range(exp_fuse):
                            j = jj * exp_fuse + u
                            for c in range(nsub):
                                cs = slice(c * nmm, (c + 1) * nmm)
                                nc.tensor.matmul(
                                    o_t[:, cs], lhsT=vp_t[:, j, :],
                                    rhs=pt_t[:, u, cs],
                                    start=(j == 0), stop=(j == j_chunks - 1))
                    if "sums" in stages:
                        if slab_adds and sum_fold == 0:
                            if first_t:
                                fold_state[(h, g)] = pt_t[:]
                            else:
                                t = fsp.tile([128, exp_fuse, sq], dt_in,
                                             name="fs")
                                nc.vector.tensor_add(t[:], fold_state[(h, g)],
                                                     pt_t[:])
                                fold_state[(h, g)] = t[:]
                            if last_t:
                                folded = fold_state.pop((h, g))
                                if exp_fuse == 1:
                                    ff = folded
                                else:
                                    fft = fsp.tile([128, sq], dt_in,
                                                   name="ff")
                                    acc = folded[:, 0, :]
                                    for u in range(1, exp_fuse):
                                        nc.vector.tensor_add(
                                            fft[:], acc, folded[:, u, :])
                                        acc = fft[:]
                                    ff = acc
                                if sm_pool:
                                    sm_t = smp.tile([128, sq], f32,
                                                    name="sm_t")
                                    sm_ap = sm_t[:]
                                else:
                                    sm_t = sp.tile([128, exp_fuse, sq], f32,
                                                   name="sm_t", tag="s_t")
                                    sm_ap = sm_t[:, 0, :]
                                for c in range(nsub):
                                    cs = slice(c * nmm, (c + 1) * nmm)
                                    nc.tensor.matmul(
                                        sm_ap[:, cs], lhsT=ones[:],
                                        rhs=ff[:, cs], start=True, stop=True)
                                group_psum[(h, g)] = (o_t, sm_ap)
                        elif sum_fold == 0:
                            gps_start = (j_chunks - (gps_adds + 1)
                                         if gps_adds else j_chunks)
                            for u in range(exp_fuse):
                                j = jj * exp_fuse + u
                                if j >= gps_start:
                                    key = (h, g, "gps")
                                    if j == gps_start:
                                        fold_state[key] = pt_t[:, u, :]
                                    else:
                                        acc = fold_state[key]
                                        t = gsp.tile([128, sq], dt_in,
                                                     name="gs")
                                        nc.gpsimd.tensor_add(t[:], acc,
                                                             pt_t[:, u, :])
                                        fold_state[key] = t[:]
                                elif j == 0:
                                    fold_state[(h, g)] = pt_t[:, u, :]
                                else:
                                    acc = fold_state[(h, g)]
                                    t = fsp.tile([128, sq], dt_in, name="fs")
                                    sw = sq // add_split
                                    for a in range(add_split):
                                        asl = slice(a * sw, (a + 1) * sw)
                                        nc.vector.tensor_add(
                                            t[:, asl], acc[:, asl],
                                            pt_t[:, u, asl])
                                    fold_state[(h, g)] = t[:]
                            if last_t:
                                facc = fold_state.pop((h, g))
                                if gps_adds:
                                    gacc = fold_state.pop((h, g, "gps"))
                                    t = fsp.tile([128, sq], dt_in, name="fs")
                                    nc.vector.tensor_add(t[:], facc, gacc)
                                    facc = t[:]
                                if sm_pool:
                                    sm_t = smp.tile([128, sq], f32,
                                                    name="sm_t")
                                    sm_ap = sm_t[:]
                                else:
                                    sm_t = sp.tile([128, exp_fuse, sq], f32,
                                                   name="sm_t", tag="s_t")
                                    sm_ap = sm_t[:, 0, :]
                                for c in range(nsub):
                                    cs = slice(c * nmm, (c + 1) * nmm)
                                    nc.tensor.matmul(
                                        sm_ap[:, cs], lhsT=ones[:],
                                        rhs=facc[:, cs],
                                        start=True, stop=True)
                                group_psum[(h, g)] = (o_t, sm_ap)
                    if last_t and "tail" in stages:
                        o_t, sm_t = group_psum[(h, g)]
                        recip_t = rcp.tile([128, sq], f32)
                        if recip == "fast":
                            nc.vector.reciprocal_approx_fast(recip_t[:],
                                                             sm_t[:])
                        else:
                            nc.vector.reciprocal(recip_t[:], sm_t[:])
                        otn = outsb.tile([D, sq], out_dt)
                        nc.vector.tensor_mul(otn[:], o_t[:], recip_t[:])
                        nc.sync.dma_start(
                            outT[h, :, g * sq:(g + 1) * sq], otn[:])
                        del group_psum[(h, g)]

                chunk_list = [(h, g, jj) for h in range(hl)
                              for g in range(groups)
                              for jj in range(jj_units)]
                pending = []
                for ch in chunk_list:
                    pending.append((ch, emit_qk(*ch)))
                    if len(pending) > lookahead:
                        (h, g, j), s_t = pending.pop(0)
                        emit_rest(h, g, j, s_t)
                while pending:
                    (h, g, j), s_t = pending.pop(0)
                    emit_rest(h, g, j, s_t)

            if repeat == 1:
                body()
            else:
                with tc.For_i(0, repeat, 1) as _i:
                    body(_i)

    nc.compile()
    return nc, np_in


def shard_inputs(q, k, v, s=S, hl=HL, ncores=NCORES, np_in=BF16):
    """Host-side layout prep: per-core per-head transposed views, cast.
    Q is pre-scaled by softmax_scale*log2(e) (see build_program)."""
    nheads = ncores * hl
    j_chunks = s // 128
    # [s,b,n,d] -> [b,n,d,s] -> [H, d, s]
    qt = np.ascontiguousarray(
        q.transpose(1, 2, 3, 0) * np.float32(SOFTMAX_SCALE * LOG2E)
    ).reshape(nheads, D, s)
    kt = np.ascontiguousarray(k.transpose(1, 2, 3, 0)).reshape(nheads, D, s)
    # [s,b,n,d] -> [b,n,s,d] -> [H, J, 128, d] -> [H, 128, J, d]
    vpm = (v.transpose(1, 2, 0, 3)
            .reshape(nheads, j_chunks, 128, D)
            .transpose(0, 2, 1, 3))
    qt = qt.astype(np_in)
    kt = kt.astype(np_in)
    vpm = np.ascontiguousarray(vpm).astype(np_in)
    in_maps = []
    for c in range(ncores):
        sl = slice(c * hl, (c + 1) * hl)
        in_maps.append({
            "qt": np.ascontiguousarray(qt[sl]),
            "kt": np.ascontiguousarray(kt[sl]),
            "vp": np.ascontiguousarray(vpm[sl]),
        })
    return in_maps


def gather_output(results, s=S, hl=HL, ncores=NCORES):
    """[{outT: [hl, D, s]}] per core -> full [s, B, NH*D] fp32."""
    outT = np.stack([np.asarray(r["outT"]).astype(np.float32)
                     for r in results])                        # [C, hl, D, s]
    out_heads = outT.reshape(ncores * hl, D, s)                # [H, D, s]
    out = out_heads.transpose(2, 0, 1)                         # [s, H, D]
    return np.ascontiguousarray(out).reshape(s, B, NH * D)


_CACHE = {}

# Measured (paired A/B on HW, reps=1 vs 4097 wall-differencing):
#   - recip="fast" (RECIPROCAL_APPROX_FAST, ~51 ULP) vs bit-exact
#     reciprocal (~6 cyc/elem): neutral-to-positive, strictly less DVE.
#   - out_dtype="bf16": -25 us/iter (halves output DMA + 16-bit writes).
#   - slab_adds / gps_adds (GpSimd offload) / dve_exp (Schraudolph exp on
#     DVE) / sq=1024 / lookahead 1 or 3 / bigger pools: all regressions.
BEST_CFG = dict(sq=512, nmm=512, lookahead=2, s_bufs=3, o_bufs=2,
                exp_fuse=2, pt_bufs=8, tail_bufs=3, fs_bufs=5, in_bufs=3,
                dve_exp=0, recip="fast", out_dtype="bf16", slab_adds=0)


def _get_program(**cfg):
    key = tuple(sorted((k, str(v)) for k, v in cfg.items()))
    if key not in _CACHE:
        _CACHE[key] = build_program(**cfg)
    return _CACHE[key]


def run(q, k, v, trace=False, **cfg):
    """Run on the 8 NeuronCores; returns (out, BassKernelResults)."""
    from concourse.bass_utils import run_bass_kernel_spmd

    full_cfg = {**BEST_CFG, **cfg}
    nc, np_in = _get_program(**full_cfg)
    in_maps = shard_inputs(q, k, v, np_in=np_in)
    res = run_bass_kernel_spmd(nc, in_maps, core_ids=list(range(NCORES)),
                               trace=trace)
    return gather_output(res.results), res


def kernel(q, k, v):
    q = np.asarray(q, dtype=np.float32)
    k = np.asarray(k, dtype=np.float32)
    v = np.asarray(v, dtype=np.float32)
    out, _ = run(q, k, v)
    return out
